# revision 1
# baseline (speedup 1.0000x reference)
"""Trainium2 Bass kernel for a 3-layer GRU (PyTorch gate order) + final FC.

Problem shapes (hardcoded): x [256, 512, 64], H=128, 3 layers, NCLASS=6.
Sharding: data-parallel over batch, 32 rows per core on 8 cores.

Per-core design:
  - Partition layout [96, *]: rows 32l:32l+32 hold layer l's batch (32 rows).
    The 3 layers run software-pipelined with a 1-slot lag (layer l processes
    timestep t at slot s = t + l), so one set of batched elementwise
    instructions covers all three layers each slot.
  - PSUM gate layout per slot (one bank, 512 f32):
      cols 0:128   = gxn  (W_in x + b_in)
      cols 128:256 = r pre-act (W_ir x + W_hr h + b_ir + b_hr)
      cols 256:384 = z pre-act
      cols 384:512 = hn   (W_hn h + b_hn)
    Biases enter via a K=3 selector matmul (E[3,96] x B3[3,512]) that also
    opens the accumulation group; the input projection (lhsT = x_t^T or
    h^{l-1}_t{}^T) and recurrent matmuls accumulate on top. The three
    layers' recurrent matmuls are column-tiled (out base partitions
    0/32/64) and run concurrently in the PE array.
  - GRU cell: r,z = sigmoid(psum), u = 1-z, q = z*h, t = r*hn,
    npre = t+gxn, n = tanh(npre), h' = u*n + q.
  - h' [96,128] is transposed each slot via the PE (identity matmul) and
    copied to SBUF as hT [128,96]; hT feeds both the next slot's recurrent
    matmuls and the next layer's input projection.
"""

import numpy as np

B, T, IN, H, NCLASS = 256, 512, 64, 128, 6
NCORES = 8
BL = B // NCORES  # 32
G3 = 3 * H  # 384
NL = 3  # layers
P = NL * BL  # 96 partitions of batch x layer
WCOLS0 = 6 * G3 + 4 * H + 2 * P + 2 * NCLASS + BL  # packed const columns
WCOLS = WCOLS0 + T * BL  # + transposed input at rows 0:64, cols WCOLS0:

_cached = {}


def _build_bass(nslots=None):
    key = ("nc", nslots)
    if key in _cached:
        return _cached[key]

    from contextlib import ExitStack

    import concourse.bass as bass
    import concourse.tile as tile
    from concourse import mybir
    from concourse.tile_rust import add_dep_helper

    f32 = mybir.dt.float32
    AF = mybir.ActivationFunctionType
    ALU = mybir.AluOpType

    nc = bass.Bass()

    # ---- DRAM I/O (per core; weights identical across cores) ----
    # weights, constants AND the transposed input all packed into one
    # [128, WCOLS] tensor: one DMA -> one semaphore -> instructions stay
    # under the ISA's tiny per-instruction sync-wait limit
    wp_d = nc.dram_tensor("wpack", [128, WCOLS], f32, kind="ExternalInput")
    out_d = nc.dram_tensor("out", [BL, NCLASS], f32, kind="ExternalOutput")

    NSLOT = (T + NL - 1) if nslots is None else nslots  # 514 full

    with ExitStack() as ctx:
        tc = ctx.enter_context(tile.TileContext(nc))
        const = ctx.enter_context(tc.tile_pool(name="const", bufs=1))
        work = ctx.enter_context(tc.tile_pool(name="work", bufs=3))
        psum = ctx.enter_context(tc.tile_pool(name="psum", bufs=4, space="PSUM"))
        psum_t = ctx.enter_context(tc.tile_pool(name="psum_t", bufs=2, space="PSUM"))
        psum_fc = ctx.enter_context(tc.tile_pool(name="psum_fc", bufs=1, space="PSUM"))

        # ---- load constants ----
        wpack = const.tile([128, WCOLS], f32)
        wp_dma = nc.sync.dma_start(out=wpack, in_=wp_d[:, :])
        xT = wpack[0:IN, WCOLS0 : WCOLS0 + T * BL]
        rih = [
            wpack[0:IN, 0:G3],
            wpack[0:H, G3 : 2 * G3],
            wpack[0:H, 2 * G3 : 3 * G3],
        ]
        rhh = [wpack[0:H, (3 + l) * G3 : (4 + l) * G3] for l in range(NL)]
        c0 = 6 * G3
        b3 = wpack[0:NL, c0 : c0 + 4 * H]
        esel = wpack[0:NL, c0 + 4 * H : c0 + 4 * H + P]
        id96 = wpack[0:P, c0 + 4 * H + P : c0 + 4 * H + 2 * P]
        c1 = c0 + 4 * H + 2 * P
        fcw = wpack[0:H, c1 : c1 + NCLASS]
        fcb = wpack[0:1, c1 + NCLASS : c1 + 2 * NCLASS]
        ones = wpack[0:1, c1 + 2 * NCLASS : c1 + 2 * NCLASS + BL]

        # ---- persistent state rings ----
        hT_ring = [const.tile([H, P], f32, tag=f"hT{i}", name=f"hT{i}") for i in range(3)]
        hB_ring = [const.tile([P, H], f32, tag=f"hB{i}", name=f"hB{i}") for i in range(2)]
        for tl in hT_ring:
            nc.vector.memset(tl, 0.0)
        for tl in hB_ring:
            nc.vector.memset(tl, 0.0)

        tr_hist = []
        for s in range(NSLOT):
            if nslots is None:
                a = max(0, s - (T - 1))  # first active layer
                b = min(NL - 1, s) + 1  # last active layer + 1
            else:
                a, b = 0, NL  # timing builds: all layers always active
            pa, pb = 32 * a, 32 * b
            hT_prev = hT_ring[(s - 1) % 3]
            hB_prev = hB_ring[(s - 1) % 2]
            hB_cur = hB_ring[s % 2]

            ps = psum.tile([P, 4 * H], f32)

            # bias matmul opens the accumulation group (full partition range:
            # matmuls with out base partition 32 may span at most 32 rows)
            bias_mm = nc.tensor.matmul(
                ps[:, :],
                esel[:, :],
                b3[:, :],
                start=True,
                stop=False,
            )
            # keep the bias matmul from floating ahead of the transpose two
            # slots back: by then the PE has already waited on recent DVE/ACT
            # ticks, so psum-recycle deps are subsumed and the matmul stays
            # under the ISA's 2-sync-wait limit
            if len(tr_hist) >= 2:
                add_dep_helper(
                    bias_mm.ins, tr_hist[-2].ins, sync=False,
                    reason="cap matmul sync waits",
                )
            # input projections (cols 0:384 = gxn|r|z)
            for l in range(a, b):
                t_l = (s - l) % T
                if l == 0:
                    lhs = xT[:, t_l * BL : (t_l + 1) * BL]
                else:
                    lhs = hT_prev[:, 32 * (l - 1) : 32 * l]
                nc.tensor.matmul(
                    ps[32 * l : 32 * (l + 1), 0:G3],
                    lhs,
                    rih[l][:, :],
                    start=False,
                    stop=False,
                )
            # recurrent matmuls (cols 128:512 = r|z|hn)
            for l in range(a, b):
                nc.tensor.matmul(
                    ps[32 * l : 32 * (l + 1), H : 4 * H],
                    hT_prev[:, 32 * l : 32 * (l + 1)],
                    rhh[l][:, :],
                    start=False,
                    stop=(l == b - 1),
                )

            rz = work.tile([P, 2 * H], f32, tag="rz")
            u = work.tile([P, H], f32, tag="u")
            q = work.tile([P, H], f32, tag="q")
            tt = work.tile([P, H], f32, tag="tt")
            npre = work.tile([P, H], f32, tag="npre")
            n = work.tile([P, H], f32, tag="n")
            w = work.tile([P, H], f32, tag="w")
            # PSUM access patterns starting at partition 32 may span at most
            # 32 partitions -> split the [32:96] ramp slot into two ranges.
            rngs = [(32, 64), (64, 96)] if (pa, pb) == (32, 96) else [(pa, pb)]
            for ra, rb in rngs:
                nc.scalar.activation(rz[ra:rb, 0:H], ps[ra:rb, H : 2 * H], AF.Sigmoid)
                nc.scalar.activation(
                    rz[ra:rb, H : 2 * H], ps[ra:rb, 2 * H : 3 * H], AF.Sigmoid
                )
                nc.vector.tensor_scalar(
                    u[ra:rb, :], rz[ra:rb, H : 2 * H], -1.0, 1.0, ALU.mult, ALU.add
                )
                nc.vector.tensor_mul(
                    q[ra:rb, :], rz[ra:rb, H : 2 * H], hB_prev[ra:rb, :]
                )
                nc.vector.tensor_mul(
                    tt[ra:rb, :], rz[ra:rb, 0:H], ps[ra:rb, 3 * H : 4 * H]
                )
                nc.vector.tensor_add(npre[ra:rb, :], tt[ra:rb, :], ps[ra:rb, 0:H])
                nc.scalar.activation(n[ra:rb, :], npre[ra:rb, :], AF.Tanh)
                nc.vector.tensor_mul(w[ra:rb, :], u[ra:rb, :], n[ra:rb, :])
                last_dve = nc.vector.tensor_add(
                    hB_cur[ra:rb, :], w[ra:rb, :], q[ra:rb, :]
                )

            # transpose h' -> hT for next slot's matmuls
            pt = psum_t.tile([H, P], f32)
            tr = nc.tensor.transpose(pt[:, :], hB_cur[:, :], id96[:, :])
            tr_hist.append(tr)
            nc.scalar.activation(hT_ring[s % 3][:, :], pt[:, :], AF.Copy)

        # ---- FC head on layer 2's final h ----
        s_last = NSLOT - 1
        pfc = psum_fc.tile([BL, NCLASS], f32)
        nc.tensor.matmul(
            pfc[:, :],
            hT_ring[s_last % 3][:, 64:96],
            fcw[:, :],
            start=True,
            stop=False,
        )
        last_pe = nc.tensor.matmul(
            pfc[:, :], ones[:, :], fcb[:, :], start=False, stop=True
        )
        out_sb = const.tile([BL, NCLASS], f32)
        last_act = nc.scalar.activation(out_sb[:, :], pfc[:, :], AF.Copy)
        out_dma = nc.sync.dma_start(out=out_d[:, :], in_=out_sb)

        # funnel all engine tails through SP nops with <=2 sync deps each, so
        # the TileContext-exit Drain needs no more than the ISA wait limit
        for dep in (last_act, last_pe, last_dve, wp_dma, out_dma):
            fn = nc.sync.nop()
            add_dep_helper(fn.ins, dep.ins, sync=True, reason="tail funnel")

    _cached[key] = nc
    return nc


def _prep_weights(w_ih0, w_ih1, w_ih2, w_hh, b_ih, b_hh, fc_w, fc_b):
    f = np.float32
    w_ih = [np.asarray(w_ih0, f), np.asarray(w_ih1, f), np.asarray(w_ih2, f)]
    w_hh = np.asarray(w_hh, f)
    b_ih = np.asarray(b_ih, f)
    b_hh = np.asarray(b_hh, f)

    wp = np.zeros((128, WCOLS), f)
    for l in range(NL):
        wi = w_ih[l]
        k = wi.shape[1]
        # psum cols 0:384 = [gxn | r | z] -> [W_in^T, W_ir^T, W_iz^T]
        wp[0:k, l * G3 : (l + 1) * G3] = np.concatenate(
            [wi[2 * H : 3 * H].T, wi[0:H].T, wi[H : 2 * H].T], axis=1
        )
        wh = w_hh[l]
        # psum cols 128:512 = [r | z | hn] -> [W_hr^T, W_hz^T, W_hn^T]
        wp[0:H, (3 + l) * G3 : (4 + l) * G3] = np.concatenate(
            [wh[0:H].T, wh[H : 2 * H].T, wh[2 * H : 3 * H].T], axis=1
        )
    c0 = 6 * G3
    for l in range(NL):
        wp[l, c0 : c0 + H] = b_ih[l, 2 * H : 3 * H]  # b_in
        wp[l, c0 + H : c0 + 2 * H] = b_ih[l, 0:H] + b_hh[l, 0:H]  # r
        wp[l, c0 + 2 * H : c0 + 3 * H] = b_ih[l, H : 2 * H] + b_hh[l, H : 2 * H]  # z
        wp[l, c0 + 3 * H : c0 + 4 * H] = b_hh[l, 2 * H : 3 * H]  # b_hn
        wp[l, c0 + 4 * H + 32 * l : c0 + 4 * H + 32 * (l + 1)] = 1.0  # esel
    wp[0:P, c0 + 4 * H + P : c0 + 4 * H + 2 * P] = np.eye(P, dtype=f)
    c1 = c0 + 4 * H + 2 * P
    wp[0:H, c1 : c1 + NCLASS] = np.asarray(fc_w, f).T
    wp[0, c1 + NCLASS : c1 + 2 * NCLASS] = np.asarray(fc_b, f)
    wp[0, c1 + 2 * NCLASS : c1 + 2 * NCLASS + BL] = 1.0  # ones
    return {"wpack": wp}


def kernel(x, w_ih0, w_ih1, w_ih2, w_hh, b_ih, b_hh, fc_w, fc_b, **_ignored):
    from concourse.bass_utils import run_bass_kernel_spmd

    x = np.asarray(x, np.float32)
    shared = _prep_weights(w_ih0, w_ih1, w_ih2, w_hh, b_ih, b_hh, fc_w, fc_b)

    in_maps = []
    for c in range(NCORES):
        xc = x[c * BL : (c + 1) * BL]  # [32, 512, 64]
        wp = shared["wpack"].copy()
        wp[0:IN, WCOLS0:] = xc.transpose(2, 1, 0).reshape(IN, T * BL)
        in_maps.append({"wpack": wp})

    nc = _build_bass()
    res = run_bass_kernel_spmd(nc, in_maps, core_ids=list(range(NCORES)))
    out = np.concatenate([r["out"] for r in res.results], axis=0)
    return out.astype(np.float32)


if __name__ == "__main__":
    rng = np.random.default_rng(0)
    ins = {
        "x": rng.standard_normal((B, T, IN), dtype=np.float32),
        "w_ih0": rng.standard_normal((G3, IN), dtype=np.float32) * 0.05,
        "w_ih1": rng.standard_normal((G3, H), dtype=np.float32) * 0.05,
        "w_ih2": rng.standard_normal((G3, H), dtype=np.float32) * 0.05,
        "w_hh": rng.standard_normal((3, G3, H), dtype=np.float32) * 0.05,
        "b_ih": rng.standard_normal((3, G3), dtype=np.float32) * 0.05,
        "b_hh": rng.standard_normal((3, G3), dtype=np.float32) * 0.05,
        "fc_w": rng.standard_normal((NCLASS, H), dtype=np.float32) * 0.05,
        "fc_b": rng.standard_normal((NCLASS,), dtype=np.float32) * 0.05,
    }
    print(kernel(**ins)[:2])



# revision 11
# speedup vs baseline: 1.8447x; 1.8447x over previous
"""Trainium2 Bass kernel for a 3-layer GRU (PyTorch gate order) + final FC.

Problem shapes (hardcoded): x [256, 512, 64], H=128, 3 layers, NCLASS=6.
Sharding: data-parallel over batch, 32 rows per core on 8 cores.

Per-core design:
  - Partition layout [96, *]: rows 32l:32l+32 hold layer l's batch (32 rows).
    The 3 layers run software-pipelined with a 1-slot lag (layer l processes
    timestep t at slot s = t + l), so one set of batched elementwise
    instructions covers all three layers each slot.
  - All matmul operands are bf16 (PSUM accumulates fp32): fp32 matmuls
    stream at 4 cycles/col (LOW_HIGH dual pass), bf16 at 1 — 4x faster PE.
    Elementwise math stays fp32; h is only rounded to bf16 on the matmul
    (hT) path.
  - TWO gate PSUM banks per slot (ScalarE+VectorE may not touch the same
    PSUM bank concurrently, so banks are split by consumer engine):
      bank A [96, 256] = r|z preacts   — read only by ACT (sigmoids)
      bank B [96, 256] = gxn|hn        — read only by DVE (tt, npre)
    Bank A's matmuls are emitted first so the sigmoid starts while bank
    B's columns are still streaming.  Biases enter via K=3 selector
    matmuls (esel[3,96] x b3A/b3B[3,256]) that open each bank's
    accumulation group.
  - GRU cell: one ACTIVATE computes sigmoid over r|z (FD=256); u = 1-z is
    computed as sigmoid(-z_pre) on ACT (exact identity), hidden behind the
    DVE's tt = r*hn, npre = tt+gxn; q = z*h hides behind the tanh; only
    w = u*n sits after the tanh on the chain.
  - hT for the next slot's matmuls is built as T(q) + T(w) with two
    accumulating PE transposes (T(q) runs during the tanh), evacuated by
    one ACT copy to bf16; hB' = w + q (fp32, for the next slot's z*h)
    runs off the critical path.
  - A 1-element DVE copy from bank B right after the matmuls makes the
    DVE engine clock observe the PE tick, so every TensorTensor needs at
    most ONE sync wait (the TT encoding has a single wait slot; matmuls
    get a second via their LDWEIGHTS).
"""

import numpy as np

B, T, IN, H, NCLASS = 256, 512, 64, 128, 6
NCORES = 8
BL = B // NCORES  # 32
G3 = 3 * H  # 384
NL = 3  # layers
P = NL * BL  # 96 partitions of batch x layer
H2 = 2 * H  # 256

# bf16 pack column layout
_OFF_RIHA = 0                      # 3 x [K_l, 256]
_OFF_RIHB = 3 * H2                 # 3 x [K_l, 128]
_OFF_RHHA = _OFF_RIHB + 3 * H      # 3 x [128, 256]
_OFF_RHHB = _OFF_RHHA + 3 * H2     # 3 x [128, 128]
_OFF_B3A = _OFF_RHHB + 3 * H       # [3, 256]
_OFF_B3B = _OFF_B3A + H2           # [3, 256]
_OFF_ESEL = _OFF_B3B + H2          # [3, 96]
_OFF_FCW = _OFF_ESEL + P           # [128, 6]
_OFF_FCB = _OFF_FCW + NCLASS       # [1, 6]
_OFF_ONES = _OFF_FCB + NCLASS      # [1, 32]
WCOLS0 = _OFF_ONES + BL
WCOLS = WCOLS0 + T * BL            # + transposed input at rows 0:64

_cached = {}


def _build_bass(nslots=None):
    key = ("nc", nslots)
    if key in _cached:
        return _cached[key]

    from contextlib import ExitStack

    import concourse.bass as bass
    import concourse.tile as tile
    from concourse import mybir
    from concourse.tile_rust import add_dep_helper

    f32 = mybir.dt.float32
    bf16 = mybir.dt.bfloat16
    AF = mybir.ActivationFunctionType

    nc = bass.Bass()

    wp_d = nc.dram_tensor("wpack", [128, WCOLS], bf16, kind="ExternalInput")
    id_d = nc.dram_tensor("idpack", [P, P], f32, kind="ExternalInput")
    out_d = nc.dram_tensor("out", [BL, NCLASS], f32, kind="ExternalOutput")

    NSLOT = (T + NL - 1) if nslots is None else nslots  # 514 full

    with ExitStack() as ctx:
        tc = ctx.enter_context(tile.TileContext(nc))
        const = ctx.enter_context(tc.tile_pool(name="const", bufs=1))
        work = ctx.enter_context(tc.tile_pool(name="work", bufs=3))
        psumA = ctx.enter_context(tc.tile_pool(name="psumA", bufs=3, space="PSUM"))
        psumB = ctx.enter_context(tc.tile_pool(name="psumB", bufs=3, space="PSUM"))
        psum_t = ctx.enter_context(tc.tile_pool(name="psum_t", bufs=1, space="PSUM"))
        psum_fc = ctx.enter_context(tc.tile_pool(name="psum_fc", bufs=1, space="PSUM"))

        # ---- load constants ----
        wpack = const.tile([128, WCOLS], bf16)
        wp_dma = nc.sync.dma_start(out=wpack, in_=wp_d[:, :])
        id96 = const.tile([P, P], f32)
        id_dma = nc.sync.dma_start(out=id96, in_=id_d[:, :])
        xT = wpack[0:IN, WCOLS0 : WCOLS0 + T * BL]
        kin = [IN, H, H]
        rihA = [
            wpack[0 : kin[l], _OFF_RIHA + l * H2 : _OFF_RIHA + (l + 1) * H2]
            for l in range(NL)
        ]
        rihB = [
            wpack[0 : kin[l], _OFF_RIHB + l * H : _OFF_RIHB + (l + 1) * H]
            for l in range(NL)
        ]
        rhhA = [
            wpack[0:H, _OFF_RHHA + l * H2 : _OFF_RHHA + (l + 1) * H2]
            for l in range(NL)
        ]
        rhhB = [
            wpack[0:H, _OFF_RHHB + l * H : _OFF_RHHB + (l + 1) * H]
            for l in range(NL)
        ]
        b3A = wpack[0:NL, _OFF_B3A : _OFF_B3A + H2]
        b3B = wpack[0:NL, _OFF_B3B : _OFF_B3B + H2]
        esel = wpack[0:NL, _OFF_ESEL : _OFF_ESEL + P]
        fcw = wpack[0:H, _OFF_FCW : _OFF_FCW + NCLASS]
        fcb = wpack[0:1, _OFF_FCB : _OFF_FCB + NCLASS]
        ones = wpack[0:1, _OFF_ONES : _OFF_ONES + BL]

        # ---- persistent state rings ----
        hT_ring = [const.tile([H, P], bf16, tag=f"hT{i}", name=f"hT{i}") for i in range(3)]
        hB_ring = [const.tile([P, H], f32, tag=f"hB{i}", name=f"hB{i}") for i in range(2)]
        memsets = []
        for tl in hT_ring:
            memsets.append(nc.vector.memset(tl, 0.0))
        for tl in hB_ring:
            memsets.append(nc.vector.memset(tl, 0.0))
        # make the DVE observe both input-DMA semaphores up front (waits are
        # not transitively subsumed across engines)
        add_dep_helper(memsets[0].ins, wp_dma.ins, sync=True, reason="observe dma")
        add_dep_helper(memsets[1].ins, id_dma.ins, sync=True, reason="observe dma")

        trw_hist = []
        for s in range(NSLOT):
            if nslots is None:
                a = max(0, s - (T - 1))  # first active layer
                b = min(NL - 1, s) + 1  # last active layer + 1
            else:
                a, b = 0, NL  # timing builds: all layers always active
            pa, pb = 32 * a, 32 * b
            hT_prev = hT_ring[(s - 1) % 3]
            hB_prev = hB_ring[(s - 1) % 2]
            hB_cur = hB_ring[s % 2]

            psA = psumA.tile([P, H2], f32)
            psB = psumB.tile([P, H2], f32)

            # bias matmuls open each bank's accumulation group (full
            # partition range: matmuls with out base partition 32 may span
            # at most 32 rows)
            biasA = nc.tensor.matmul(
                psA[:, :], esel[:, :], b3A[:, :], start=True, stop=False
            )
            biasB = nc.tensor.matmul(
                psB[:, :], esel[:, :], b3B[:, :], start=True, stop=False
            )
            if s == 0:
                # PE observes the id96 DMA before the first transpose needs
                # it (slot-0 biasB otherwise has no waits of its own)
                add_dep_helper(biasB.ins, id_dma.ins, sync=True, reason="observe dma")
            # keep the bias matmuls from floating ahead of the transpose two
            # slots back (psum-recycle deps then stay subsumed and the
            # matmuls keep <=1 sync wait + the ordering dep)
            if len(trw_hist) >= 2:
                add_dep_helper(
                    biasA.ins, trw_hist[-2].ins, sync=False,
                    reason="cap matmul sync waits",
                )
                add_dep_helper(
                    biasB.ins, trw_hist[-2].ins, sync=False,
                    reason="cap matmul sync waits",
                )

            def lhs_for(l, s=s, hT_prev=hT_prev):
                if l == 0:
                    t_l = s % T
                    return xT[:, t_l * BL : (t_l + 1) * BL]
                return hT_prev[:, 32 * (l - 1) : 32 * l]

            # bank A first: input rz projections, then recurrent rz
            for l in range(a, b):
                nc.tensor.matmul(
                    psA[32 * l : 32 * (l + 1), :],
                    lhs_for(l),
                    rihA[l][:, :],
                    start=False,
                    stop=False,
                )
            for l in range(a, b):
                nc.tensor.matmul(
                    psA[32 * l : 32 * (l + 1), :],
                    hT_prev[:, 32 * l : 32 * (l + 1)],
                    rhhA[l][:, :],
                    start=False,
                    stop=(l == b - 1),
                )
            # bank B: gxn input projection (cols 0:128), hn recurrent (128:256)
            for l in range(a, b):
                nc.tensor.matmul(
                    psB[32 * l : 32 * (l + 1), 0:H],
                    lhs_for(l),
                    rihB[l][:, :],
                    start=False,
                    stop=False,
                )
            for l in range(a, b):
                nc.tensor.matmul(
                    psB[32 * l : 32 * (l + 1), H:H2],
                    hT_prev[:, 32 * l : 32 * (l + 1)],
                    rhhB[l][:, :],
                    start=False,
                    stop=(l == b - 1),
                )

            rz = work.tile([P, H2], f32, tag="rz")
            u = work.tile([P, H], f32, tag="u")
            q = work.tile([P, H], f32, tag="q")
            tt = work.tile([P, H], f32, tag="tt")
            npre = work.tile([P, H], f32, tag="npre")
            n = work.tile([P, H], f32, tag="n")
            w = work.tile([P, H], f32, tag="w")
            dob = work.tile([1, 1], f32, tag="dob")
            dob2 = work.tile([1, 1], f32, tag="dob2")
            # DVE observes the PE tick (bank B is DVE-only, so no cross-
            # engine PSUM-bank serialization is introduced)
            nc.vector.tensor_copy(dob, psB[pa : pa + 1, H : H + 1])
            # PSUM access patterns starting at partition 32 may span at most
            # 32 partitions -> split the [32:96] ramp slot into two ranges.
            rngs = [(32, 64), (64, 96)] if (pa, pb) == (32, 96) else [(pa, pb)]
            for ra, rb in rngs:
                # r|z in one ACTIVATE (FD=256)
                nc.scalar.activation(rz[ra:rb, :], psA[ra:rb, :], AF.Sigmoid)
                # DVE observes the sigmoid's ACT tick via a 1-element SBUF
                # copy; tt then carries only the PSUM bank-chain wait
                nc.vector.tensor_copy(dob2, rz[ra : ra + 1, 0:1])
                nc.vector.tensor_mul(
                    tt[ra:rb, :], rz[ra:rb, 0:H], psB[ra:rb, H:H2]
                )
                # u = 1 - z = sigmoid(-z_pre), on ACT while the DVE runs
                nc.scalar.activation(
                    u[ra:rb, :], psA[ra:rb, H:H2], AF.Sigmoid, scale=-1.0
                )
                nc.vector.tensor_add(npre[ra:rb, :], tt[ra:rb, :], psB[ra:rb, 0:H])
                nc.scalar.activation(n[ra:rb, :], npre[ra:rb, :], AF.Tanh)
                nc.vector.tensor_mul(
                    q[ra:rb, :], rz[ra:rb, H:H2], hB_prev[ra:rb, :]
                )
                nc.vector.tensor_mul(w[ra:rb, :], u[ra:rb, :], n[ra:rb, :])
                last_dve = nc.vector.tensor_add(
                    hB_cur[ra:rb, :], w[ra:rb, :], q[ra:rb, :]
                )

            # hT' = T(q) + T(w) via two accumulating PE transposes; T(q)
            # runs during the tanh, so only T(w) + the copy sit on the
            # critical path.
            pt = psum_t.tile([H, P], f32)
            nc.tensor.matmul(
                pt[:, :], q[:, :], id96[:, :], is_transpose=True,
                start=True, stop=False,
            )
            trw = nc.tensor.matmul(
                pt[:, :], w[:, :], id96[:, :], is_transpose=True,
                start=False, stop=True,
            )
            trw_hist.append(trw)
            nc.scalar.activation(hT_ring[s % 3][:, :], pt[:, :], AF.Copy)

        # ---- FC head on layer 2's final h ----
        s_last = NSLOT - 1
        pfc = psum_fc.tile([BL, NCLASS], f32)
        nc.tensor.matmul(
            pfc[:, :],
            hT_ring[s_last % 3][:, 64:96],
            fcw[:, :],
            start=True,
            stop=False,
        )
        last_pe = nc.tensor.matmul(
            pfc[:, :], ones[:, :], fcb[:, :], start=False, stop=True
        )
        out_sb = const.tile([BL, NCLASS], f32)
        last_act = nc.scalar.activation(out_sb[:, :], pfc[:, :], AF.Copy)
        out_dma = nc.sync.dma_start(out=out_d[:, :], in_=out_sb)

        # funnel all engine tails through SP nops with <=2 sync deps each, so
        # the TileContext-exit Drain needs no more than the ISA wait limit
        for dep in (last_act, last_pe, last_dve, wp_dma, id_dma, out_dma):
            fn = nc.sync.nop()
            add_dep_helper(fn.ins, dep.ins, sync=True, reason="tail funnel")

    _cached[key] = nc
    return nc


def _prep_weights(w_ih0, w_ih1, w_ih2, w_hh, b_ih, b_hh, fc_w, fc_b):
    f = np.float32
    w_ih = [np.asarray(w_ih0, f), np.asarray(w_ih1, f), np.asarray(w_ih2, f)]
    w_hh = np.asarray(w_hh, f)
    b_ih = np.asarray(b_ih, f)
    b_hh = np.asarray(b_hh, f)

    wp = np.zeros((128, WCOLS), f)
    for l in range(NL):
        wi = w_ih[l]
        k = wi.shape[1]
        # bank A = [r | z]; bank B = [gxn | hn]
        wp[0:k, _OFF_RIHA + l * H2 : _OFF_RIHA + (l + 1) * H2] = np.concatenate(
            [wi[0:H].T, wi[H : 2 * H].T], axis=1
        )
        wp[0:k, _OFF_RIHB + l * H : _OFF_RIHB + (l + 1) * H] = wi[2 * H : 3 * H].T
        wh = w_hh[l]
        wp[0:H, _OFF_RHHA + l * H2 : _OFF_RHHA + (l + 1) * H2] = np.concatenate(
            [wh[0:H].T, wh[H : 2 * H].T], axis=1
        )
        wp[0:H, _OFF_RHHB + l * H : _OFF_RHHB + (l + 1) * H] = wh[2 * H : 3 * H].T
        # biases: A = [b_r | b_z] (ih+hh), B = [b_in | b_hn]
        wp[l, _OFF_B3A : _OFF_B3A + H] = b_ih[l, 0:H] + b_hh[l, 0:H]
        wp[l, _OFF_B3A + H : _OFF_B3A + H2] = (
            b_ih[l, H : 2 * H] + b_hh[l, H : 2 * H]
        )
        wp[l, _OFF_B3B : _OFF_B3B + H] = b_ih[l, 2 * H : 3 * H]
        wp[l, _OFF_B3B + H : _OFF_B3B + H2] = b_hh[l, 2 * H : 3 * H]
        wp[l, _OFF_ESEL + 32 * l : _OFF_ESEL + 32 * (l + 1)] = 1.0
    wp[0:H, _OFF_FCW : _OFF_FCW + NCLASS] = np.asarray(fc_w, f).T
    wp[0, _OFF_FCB : _OFF_FCB + NCLASS] = np.asarray(fc_b, f)
    wp[0, _OFF_ONES : _OFF_ONES + BL] = 1.0
    return {"wpack": wp}


def kernel(x, w_ih0, w_ih1, w_ih2, w_hh, b_ih, b_hh, fc_w, fc_b, **_ignored):
    import ml_dtypes

    from concourse.bass_utils import run_bass_kernel_spmd

    bf16 = ml_dtypes.bfloat16
    x = np.asarray(x, np.float32)
    shared = _prep_weights(w_ih0, w_ih1, w_ih2, w_hh, b_ih, b_hh, fc_w, fc_b)
    idp = np.eye(P, dtype=np.float32)

    in_maps = []
    for c in range(NCORES):
        xc = x[c * BL : (c + 1) * BL]  # [32, 512, 64]
        wp = shared["wpack"].copy()
        wp[0:IN, WCOLS0:] = xc.transpose(2, 1, 0).reshape(IN, T * BL)
        in_maps.append({"wpack": wp.astype(bf16), "idpack": idp})

    nc = _build_bass()
    res = run_bass_kernel_spmd(nc, in_maps, core_ids=list(range(NCORES)))
    out = np.concatenate([r["out"] for r in res.results], axis=0)
    return out.astype(np.float32)


if __name__ == "__main__":
    rng = np.random.default_rng(0)
    ins = {
        "x": rng.standard_normal((B, T, IN), dtype=np.float32),
        "w_ih0": rng.standard_normal((G3, IN), dtype=np.float32) * 0.05,
        "w_ih1": rng.standard_normal((G3, H), dtype=np.float32) * 0.05,
        "w_ih2": rng.standard_normal((G3, H), dtype=np.float32) * 0.05,
        "w_hh": rng.standard_normal((3, G3, H), dtype=np.float32) * 0.05,
        "b_ih": rng.standard_normal((3, G3), dtype=np.float32) * 0.05,
        "b_hh": rng.standard_normal((3, G3), dtype=np.float32) * 0.05,
        "fc_w": rng.standard_normal((NCLASS, H), dtype=np.float32) * 0.05,
        "fc_b": rng.standard_normal((NCLASS,), dtype=np.float32) * 0.05,
    }
    print(kernel(**ins)[:2])
